# revision 15
# baseline (speedup 1.0000x reference)
"""Trainium2 Bass kernel for BatchedActivationCSA.

Math: per token vector x (1024-dim):
    z   = FWHT(permute(x * signs))[:64]          (linear -> 64x1024 matrix A)
    g   = gate * z
    sp  = keep g_i iff |g_i| in top-16 of |g| AND |g_i| >= tau
    r   = permute^-1(FWHT(pad_64->1024(alpha*sp))) * signs   (linear -> A^T)
    out = x + r

Device kernel (per core, 2048 tokens, all d-major / "transposed" layout):
    GT  = A1-chunks.T @ X^T        [64, tok] PSUM accumulation, A1 = diag(g)@A
    G   = DVE-transpose(GT)        [tok, 64] per 128-token group
    SP  = (G^2 >= max(16th-max(G^2), tau^2)) * G     (Max8+MatchReplace+Max8)
    OUT^T = X^T + A2-chunks.T @ SP^T   with A2 = alpha * A

Host uploads x^T fp16 (no device input transposes, half the fp32 HBM
traffic), downloads out^T fp16. Ranking by squares lets GpSimd (no PSUM
access) run the threshold/select ops from SBUF copies. Residual adds are
split DVE-add / PE identity-matmul / ACT-copy+GpSimd-add to balance
engines. Precision ~1.3e-3 max rel err (gate 2e-2).

Sharding: 8 cores, core c handles batch b=c//2, seq half c%2 -> 2048 tokens.
"""

import numpy as np

BSZ, SEQ, DIM = 4, 4096, 1024
M = 64            # measure dim
NCORES = 8
TOK = BSZ * SEQ // NCORES      # 2048 tokens per core
BLK = 512                      # tokens per block
NB = TOK // BLK                # 4 blocks
NC_ = 8                        # d-chunks of 128

_cache = {}


def _fwht(y):
    """Walsh-Hadamard over last dim, identical ordering to the reference."""
    n = y.shape[-1]
    lead = y.shape[:-1]
    out = y.copy()
    h = 1
    while h < n:
        out = out.reshape(*lead, -1, 2, h)
        a, b = out[..., 0, :], out[..., 1, :]
        out = np.concatenate((a + b, a - b), axis=-1).reshape(*lead, n)
        h *= 2
    return out * (n ** -0.5)


def _build_nc():
    import concourse.bass as bass
    import concourse.mybir as mybir
    from concourse.tile import TileContext
    from concourse.masks import make_identity

    f32 = mybir.dt.float32
    f16 = mybir.dt.float16
    ACT = mybir.ActivationFunctionType
    ALU = mybir.AluOpType

    nc = bass.Bass()

    xt_d = nc.dram_tensor("xt", [NB * 128, NC_ * BLK], f16, kind="ExternalInput")
    a1t_d = nc.dram_tensor("a1t", [128, NC_ * M], f16, kind="ExternalInput")
    a2_d = nc.dram_tensor("a2", [M, DIM], f16, kind="ExternalInput")
    tau_d = nc.dram_tensor("tau", [128, 1], f32, kind="ExternalInput")
    od_d = nc.dram_tensor("od", [NB * 128, NC_ * BLK], f16, kind="ExternalOutput")

    xv = xt_d[:, :].rearrange("(b p) f -> b p f", p=128)
    ov = od_d[:, :].rearrange("(b p) f -> b p f", p=128)

    with TileContext(nc) as tc:
        with (
            tc.tile_pool(name="const", bufs=1) as consts,
            tc.tile_pool(name="xin", bufs=NB) as xin_pool,
            tc.tile_pool(name="oout", bufs=2) as out_pool,
            tc.tile_pool(name="gts", bufs=2) as gts_pool,
            tc.tile_pool(name="st", bufs=2) as st_pool,
            tc.tile_pool(name="small", bufs=8) as small,
            tc.tile_pool(name="rtp", bufs=3) as rt_pool,
            tc.tile_pool(name="ps_gt", bufs=1, space="PSUM") as ps_gt,
            tc.tile_pool(name="ps_t", bufs=2, space="PSUM") as ps_t,
            tc.tile_pool(name="ps_s", bufs=2, space="PSUM") as ps_s,
            tc.tile_pool(name="ps_o", bufs=3, space="PSUM") as ps_o,
        ):
            a1t_s = consts.tile([128, NC_ * M], f16)
            nc.scalar.dma_start(a1t_s, a1t_d[:, :])
            a2_s = consts.tile([M, DIM], f16)
            nc.scalar.dma_start(a2_s, a2_d[:, :])
            tau_s = consts.tile([128, 1], f32)
            nc.scalar.dma_start(tau_s, tau_d[:, :])
            ident16 = consts.tile([128, 128], f16)
            make_identity(nc, ident16)

            # stream all input blocks up front; they drain at line rate
            xts = []
            for b in range(NB):
                xt_s = xin_pool.tile([128, NC_ * BLK], f16, tag="x")
                nc.sync.dma_start(xt_s, xv[b])
                xts.append(xt_s)

            # keep PE busy (HAM warm) while the first block streams in
            warm = ps_o.tile([128, BLK], f32, tag="op")
            for _ in range(40):
                nc.tensor.matmul(warm[:, 0:128], lhsT=ident16, rhs=ident16,
                                 start=True, stop=True)

            # residual-add engine assignment per chunk: DVE direct-add,
            # PE identity-matmul accumulate + ACT drain, ACT copy + GpS add
            modes = ["dve", "pe", "gps", "dve", "pe", "pe", "gps", "pe"]

            def emit_mm1(b):
                """G^T = sum_c a1t_c.T @ x^T_c, then fp16 drain."""
                xt_s = xts[b]
                gtp = ps_gt.tile([M, BLK], f32, tag="gt")
                for c in range(NC_):
                    nc.tensor.matmul(
                        gtp,
                        lhsT=a1t_s[:, c * M:(c + 1) * M],
                        rhs=xt_s[:, c * BLK:(c + 1) * BLK],
                        start=(c == 0),
                        stop=(c == NC_ - 1),
                    )
                gt_s = gts_pool.tile([M, BLK], f16, tag="gts")
                nc.scalar.activation(gt_s, gtp, ACT.Copy)
                return gt_s

            def emit_spT(sps):
                """sp transposes (PE) + one st4 drain (ACT)."""
                stp4 = ps_s.tile([M, BLK], f16, tag="stp")
                for g4 in range(4):
                    nc.tensor.transpose(
                        stp4[:, g4 * 128:(g4 + 1) * 128], sps[g4], ident16
                    )
                st4 = st_pool.tile([M, BLK], f16, tag="st4")
                nc.scalar.activation(st4, stp4, ACT.Copy)
                return st4

            def emit_gT(b, gt_s):
                """G^T -> G transposes, all 4 groups into one PSUM tile."""
                gsp4 = ps_t.tile([128, 4 * M], f16, tag="gsp")
                for g4 in range(4):
                    nc.tensor.transpose(
                        gsp4[:, g4 * M:(g4 + 1) * M],
                        gt_s[:, g4 * 128:(g4 + 1) * 128],
                        ident16[0:M, 0:M],
                    )
                return gsp4

            def emit_mm2(b, st4):
                """mm2 + residual add + store for block b."""
                xt_s = xts[b]
                ot = out_pool.tile([128, NC_ * BLK], f16, tag="o")
                for c in range(NC_):
                    mode = modes[c]
                    op = ps_o.tile([128, BLK], f32, tag="op")
                    osl = ot[:, c * BLK:(c + 1) * BLK]
                    xsl = xt_s[:, c * BLK:(c + 1) * BLK]
                    if mode == "pe":
                        # x accumulates first (no st4 dep), a2 matmul second
                        nc.tensor.matmul(op, lhsT=ident16, rhs=xsl,
                                         start=True, stop=False)
                        nc.tensor.matmul(
                            op, lhsT=a2_s[:, c * 128:(c + 1) * 128],
                            rhs=st4, start=False, stop=True,
                        )
                        nc.scalar.activation(osl, op, ACT.Copy)
                    else:
                        nc.tensor.matmul(
                            op, lhsT=a2_s[:, c * 128:(c + 1) * 128],
                            rhs=st4, start=True, stop=True,
                        )
                        if mode == "dve":
                            nc.vector.tensor_tensor(osl, op, xsl, ALU.add)
                        else:
                            rtmp = rt_pool.tile([128, BLK], f16, tag="rt")
                            nc.scalar.activation(rtmp, op, ACT.Copy)
                            nc.gpsimd.tensor_tensor(osl, rtmp, xsl, ALU.add)
                nc.scalar.dma_start(ov[b], ot)

            def emit_shrink(b, gsp4):
                """|G| -> 16th max -> threshold mask -> sp tiles."""
                ag4 = small.tile([128, 4 * M], f16, tag="ag4")
                nc.scalar.activation(ag4, gsp4, ACT.Abs)
                sps = []
                for g4 in range(4):
                    gsp = gsp4[:, g4 * M:(g4 + 1) * M]
                    ag = ag4[:, g4 * M:(g4 + 1) * M]
                    m8a = small.tile([128, 8], f16, tag="m8a")
                    nc.vector.max(m8a, ag)
                    agr = small.tile([128, M], f16, tag="agr")
                    nc.vector.match_replace(agr, m8a, ag, -1.0)
                    m8b = small.tile([128, 8], f16, tag="m8b")
                    nc.vector.max(m8b, agr)
                    thr = small.tile([128, 1], f32, tag="thr")
                    nc.gpsimd.tensor_single_scalar(
                        thr, m8b[:, 7:8], tau_s[:, 0:1], ALU.max
                    )
                    # sp = (|G| >= thr) * G, fp16
                    sp = small.tile([128, M], f16, tag="sp")
                    nc.vector.scalar_tensor_tensor(
                        sp, ag, thr[:, 0:1], gsp, ALU.is_ge, ALU.mult
                    )
                    sps.append(sp)
                return sps

            # one-block-deep software pipeline. PE stream per iteration:
            # mm1(b) | spT(b-1) | gT(b) | mm2(b-1) -- each segment's operands
            # were produced at least half an iteration earlier, so the PE
            # never stalls on the cross-engine shrink chain.
            pend = None
            for b in range(NB):
                gt_s = emit_mm1(b)
                st4 = emit_spT(pend) if pend is not None else None
                gsp4 = emit_gT(b, gt_s)
                if pend is not None:
                    emit_mm2(b - 1, st4)
                pend = emit_shrink(b, gsp4)
            st4 = emit_spT(pend)
            emit_mm2(NB - 1, st4)

    _split_multi_waits(nc, mybir)
    return nc


def _split_multi_waits(nc, mybir):
    """walrus codegen allows only one sync wait on most compute instruction
    structs (PE LDWEIGHTS, DVE TS, ...). Move the waits of any multi-wait
    compute instruction onto a NoOp inserted just before it: each engine's
    sequencer executes in order, so all waits still happen-before it."""
    skip = (
        mybir.InstNoOp,
        mybir.InstEventSemaphore,
        mybir.InstUnconditionalBranch,
        mybir.InstRegisterMove,
    )
    for f in nc.m.functions:
        for blk in f.blocks:
            insts = list(blk.instructions)
            out = []
            changed = False
            for ins in insts:
                si = getattr(ins, "sync_info", None)
                if (
                    not isinstance(ins, skip)
                    and getattr(ins, "engine", None) is not None
                    and si is not None
                    and si.on_wait
                    and len(si.on_wait) > 1
                ):
                    waits = list(si.on_wait)
                    for k, w in enumerate(waits[:-1]):
                        nop = mybir.InstNoOp(
                            name=f"{ins.name}-waitsplit{k}", ins=[], outs=[]
                        )
                        nop.engine = ins.engine
                        nop.sync_info = mybir.SyncInfo(
                            on_wait=[w], on_update=[]
                        )
                        out.append(nop)
                    ins.sync_info = mybir.SyncInfo(
                        on_wait=[waits[-1]], on_update=list(si.on_update)
                    )
                    changed = True
                out.append(ins)
            if changed:
                blk.instructions = out


def _prep_inputs(x, gates, alpha, tau, signs, perm, inv_perm, target_idx):
    """Host-side prep: shard + transpose + cast per core."""
    tidx = int(target_idx)
    signs = np.asarray(signs, dtype=np.float64)
    perm = np.asarray(perm, dtype=np.int64)
    inv_perm = np.asarray(inv_perm, dtype=np.int64)
    x = np.asarray(x)

    # Sense matrix A: row i = i-th output of FWHT(permute(e * signs))[:64].
    eye = np.eye(DIM, dtype=np.float64)
    A = _fwht((eye * signs[None, :])[:, perm])[:, :M].T          # [64, 1024]
    # Reconstruct matrix B (== A, but built independently for safety)
    pad = np.zeros((M, DIM), dtype=np.float64)
    pad[:, :M] = np.eye(M)
    B = _fwht(pad)[:, inv_perm] * signs[None, :]                 # [64, 1024]

    in_maps = []
    for c in range(NCORES):
        b, half = divmod(c, 2)
        g = np.asarray(gates, dtype=np.float64)[b, tidx]         # [64]
        al = float(np.asarray(alpha, dtype=np.float64)[b, tidx, 0])
        tu = abs(float(np.asarray(tau, dtype=np.float64)[b, tidx, 0]))
        a1 = g[:, None] * A                                      # [64, 1024]
        a1t = np.ascontiguousarray(
            a1.T.reshape(NC_, 128, M).transpose(1, 0, 2).reshape(128, NC_ * M)
        ).astype(np.float16)
        a2 = (al * B).astype(np.float16)                         # [64, 1024]
        xs = x[b, half * TOK:(half + 1) * TOK, :].astype(np.float16)
        # [tok, dim] -> [blk, p, c, t] -> [NB*128, NC_*BLK]
        xt = np.ascontiguousarray(
            xs.reshape(NB, BLK, NC_, 128).transpose(0, 3, 2, 1)
        ).reshape(NB * 128, NC_ * BLK)
        in_maps.append({
            "xt": xt,
            "a1t": a1t,
            "a2": np.ascontiguousarray(a2),
            "tau": np.full((128, 1), tu, dtype=np.float32),
        })
    return in_maps


def _get_nc():
    if "nc" not in _cache:
        _cache["nc"] = _build_nc()
    return _cache["nc"]


def kernel(x, gates, alpha, tau, signs, perm, inv_perm, target_idx,
           _trace=False, _tmpdir=None):
    from concourse.bass_utils import run_bass_kernel_spmd

    nc = _get_nc()
    in_maps = _prep_inputs(x, gates, alpha, tau, signs, perm, inv_perm,
                           target_idx)
    res = run_bass_kernel_spmd(
        nc, in_maps, core_ids=list(range(NCORES)),
        trace=_trace, tmpdir=_tmpdir,
    )
    if _trace:
        _cache["last_results"] = res
    out = np.empty((BSZ, SEQ, DIM), dtype=np.float32)
    for c in range(NCORES):
        b, half = divmod(c, 2)
        od = res.results[c]["od"]
        # [NB*128, NC_*BLK] -> [blk, p, c, t] -> [tok, dim]
        o = od.reshape(NB, 128, NC_, BLK).transpose(0, 3, 2, 1).reshape(
            TOK, DIM)
        out[b, half * TOK:(half + 1) * TOK, :] = o.astype(np.float32)
    return out


# revision 17
# speedup vs baseline: 1.1440x; 1.1440x over previous
"""Trainium2 Bass kernel for BatchedActivationCSA.

Math: per token vector x (1024-dim):
    z   = FWHT(permute(x * signs))[:64]          (linear -> 64x1024 matrix A)
    g   = gate * z
    sp  = keep g_i iff |g_i| in top-16 of |g| AND |g_i| >= tau
    r   = permute^-1(FWHT(pad_64->1024(alpha*sp))) * signs   (linear -> A^T)
    out = x + r

Device kernel (per core, 2048 tokens, all d-major / "transposed" layout):
    GT  = A1-chunks.T @ X^T        [64, tok] PSUM accumulation, A1 = diag(g)@A
    G   = DVE-transpose(GT)        [tok, 64] per 128-token group
    SP  = (G^2 >= max(16th-max(G^2), tau^2)) * G     (Max8+MatchReplace+Max8)
    OUT^T = X^T + A2-chunks.T @ SP^T   with A2 = alpha * A

Host uploads x^T fp16 (no device input transposes, half the fp32 HBM
traffic), downloads out^T fp16. Ranking by squares lets GpSimd (no PSUM
access) run the threshold/select ops from SBUF copies. Residual adds are
split DVE-add / PE identity-matmul / ACT-copy+GpSimd-add to balance
engines. Precision ~1.3e-3 max rel err (gate 2e-2).

Sharding: 8 cores, core c handles batch b=c//2, seq half c%2 -> 2048 tokens.
"""

import numpy as np

BSZ, SEQ, DIM = 4, 4096, 1024
M = 64            # measure dim
NCORES = 8
TOK = BSZ * SEQ // NCORES      # 2048 tokens per core
BLK = 512                      # tokens per block
NB = TOK // BLK                # 4 blocks
NC_ = 8                        # d-chunks of 128

_cache = {}


def _fwht(y):
    """Walsh-Hadamard over last dim, identical ordering to the reference."""
    n = y.shape[-1]
    lead = y.shape[:-1]
    out = y.copy()
    h = 1
    while h < n:
        out = out.reshape(*lead, -1, 2, h)
        a, b = out[..., 0, :], out[..., 1, :]
        out = np.concatenate((a + b, a - b), axis=-1).reshape(*lead, n)
        h *= 2
    return out * (n ** -0.5)


def _build_nc():
    import concourse.bass as bass
    import concourse.mybir as mybir
    from concourse.tile import TileContext
    from concourse.masks import make_identity

    f32 = mybir.dt.float32
    f16 = mybir.dt.float16
    ACT = mybir.ActivationFunctionType
    ALU = mybir.AluOpType

    nc = bass.Bass()

    xt_d = nc.dram_tensor("xt", [NB * 128, NC_ * BLK], f16, kind="ExternalInput")
    a1t_d = nc.dram_tensor("a1t", [128, NC_ * M], f16, kind="ExternalInput")
    a2_d = nc.dram_tensor("a2", [M, DIM], f16, kind="ExternalInput")
    tau_d = nc.dram_tensor("tau", [128, 1], f32, kind="ExternalInput")
    od_d = nc.dram_tensor("od", [NB * 128, NC_ * BLK], f16, kind="ExternalOutput")

    xv = xt_d[:, :].rearrange("(b p) f -> b p f", p=128)
    ov = od_d[:, :].rearrange("(b p) f -> b p f", p=128)

    with TileContext(nc) as tc:
        with (
            tc.tile_pool(name="const", bufs=1) as consts,
            tc.tile_pool(name="xin", bufs=NB) as xin_pool,
            tc.tile_pool(name="oout", bufs=3) as out_pool,
            tc.tile_pool(name="gts", bufs=2) as gts_pool,
            tc.tile_pool(name="st", bufs=3) as st_pool,
            tc.tile_pool(name="small", bufs=14) as small,
            tc.tile_pool(name="rtp", bufs=3) as rt_pool,
            tc.tile_pool(name="ps_gt", bufs=1, space="PSUM") as ps_gt,
            tc.tile_pool(name="ps_t", bufs=2, space="PSUM") as ps_t,
            tc.tile_pool(name="ps_s", bufs=2, space="PSUM") as ps_s,
            tc.tile_pool(name="ps_o", bufs=3, space="PSUM") as ps_o,
        ):
            a1t_s = consts.tile([128, NC_ * M], f16)
            nc.scalar.dma_start(a1t_s, a1t_d[:, :])
            a2_s = consts.tile([M, DIM], f16)
            nc.scalar.dma_start(a2_s, a2_d[:, :])
            tau_s = consts.tile([128, 1], f32)
            nc.scalar.dma_start(tau_s, tau_d[:, :])
            ident16 = consts.tile([128, 128], f16)
            make_identity(nc, ident16)

            # stream all input blocks up front; they drain at line rate
            xts = []
            for b in range(NB):
                xt_s = xin_pool.tile([128, NC_ * BLK], f16, tag="x")
                nc.sync.dma_start(xt_s, xv[b])
                xts.append(xt_s)

            # keep PE busy (HAM warm) while the first block streams in
            warm = ps_o.tile([128, BLK], f32, tag="op")
            for _ in range(40):
                nc.tensor.matmul(warm[:, 0:128], lhsT=ident16, rhs=ident16,
                                 start=True, stop=True)

            # residual-add engine assignment per chunk: DVE direct-add,
            # PE identity-matmul accumulate + ACT drain, ACT copy + GpS add
            modes = ["dve", "pe", "gps", "dve", "pe", "pe", "gps", "pe"]

            def emit_mm1(b):
                """G^T = sum_c a1t_c.T @ x^T_c, then fp16 drain."""
                xt_s = xts[b]
                gtp = ps_gt.tile([M, BLK], f32, tag="gt")
                for c in range(NC_):
                    nc.tensor.matmul(
                        gtp,
                        lhsT=a1t_s[:, c * M:(c + 1) * M],
                        rhs=xt_s[:, c * BLK:(c + 1) * BLK],
                        start=(c == 0),
                        stop=(c == NC_ - 1),
                    )
                gt_s = gts_pool.tile([M, BLK], f16, tag="gts")
                nc.scalar.activation(gt_s, gtp, ACT.Copy)
                return gt_s

            def emit_spT(sps):
                """sp transposes (PE) + one st4 drain (ACT)."""
                stp4 = ps_s.tile([M, BLK], f16, tag="stp")
                for g4 in range(4):
                    nc.tensor.transpose(
                        stp4[:, g4 * 128:(g4 + 1) * 128], sps[g4], ident16
                    )
                st4 = st_pool.tile([M, BLK], f16, tag="st4")
                nc.vector.tensor_copy(st4, stp4)
                return st4

            def emit_gT(b, gt_s):
                """G^T -> G transposes, all 4 groups into one PSUM tile."""
                gsp4 = ps_t.tile([128, 4 * M], f16, tag="gsp")
                for g4 in range(4):
                    nc.tensor.transpose(
                        gsp4[:, g4 * M:(g4 + 1) * M],
                        gt_s[:, g4 * 128:(g4 + 1) * 128],
                        ident16[0:M, 0:M],
                    )
                return gsp4

            def emit_mm2(b, st4):
                """mm2 + residual add + store for block b."""
                xt_s = xts[b]
                ot = out_pool.tile([128, NC_ * BLK], f16, tag="o")
                for c in range(NC_):
                    mode = modes[c]
                    op = ps_o.tile([128, BLK], f32, tag="op")
                    osl = ot[:, c * BLK:(c + 1) * BLK]
                    xsl = xt_s[:, c * BLK:(c + 1) * BLK]
                    if mode == "pe":
                        # x accumulates first (no st4 dep), a2 matmul second
                        nc.tensor.matmul(op, lhsT=ident16, rhs=xsl,
                                         start=True, stop=False)
                        nc.tensor.matmul(
                            op, lhsT=a2_s[:, c * 128:(c + 1) * 128],
                            rhs=st4, start=False, stop=True,
                        )
                        nc.scalar.activation(osl, op, ACT.Copy)
                    else:
                        nc.tensor.matmul(
                            op, lhsT=a2_s[:, c * 128:(c + 1) * 128],
                            rhs=st4, start=True, stop=True,
                        )
                        if mode == "dve":
                            nc.vector.tensor_tensor(osl, op, xsl, ALU.add)
                        else:
                            rtmp = rt_pool.tile([128, BLK], f16, tag="rt")
                            nc.scalar.activation(rtmp, op, ACT.Copy)
                            nc.gpsimd.tensor_tensor(osl, rtmp, xsl, ALU.add)
                nc.scalar.dma_start(ov[b], ot)

            def emit_shrink(b, gsp4):
                """|G| -> 16th max -> threshold mask -> sp tiles."""
                ag4 = small.tile([128, 4 * M], f16, tag="ag4")
                nc.scalar.activation(ag4, gsp4, ACT.Abs)
                sps = []
                for g4 in range(4):
                    gsp = gsp4[:, g4 * M:(g4 + 1) * M]
                    ag = ag4[:, g4 * M:(g4 + 1) * M]
                    m8a = small.tile([128, 8], f16, tag="m8a")
                    nc.vector.max(m8a, ag)
                    agr = small.tile([128, M], f16, tag="agr")
                    nc.vector.match_replace(agr, m8a, ag, -1.0)
                    m8b = small.tile([128, 8], f16, tag="m8b")
                    nc.vector.max(m8b, agr)
                    thr = small.tile([128, 1], f32, tag="thr")
                    nc.gpsimd.tensor_single_scalar(
                        thr, m8b[:, 7:8], tau_s[:, 0:1], ALU.max
                    )
                    # sp = (|G| >= thr) * G, fp16
                    sp = small.tile([128, M], f16, tag="sp")
                    nc.vector.scalar_tensor_tensor(
                        sp, ag, thr[:, 0:1], gsp, ALU.is_ge, ALU.mult
                    )
                    sps.append(sp)
                return sps

            # two-block-deep software pipeline. PE stream per iteration:
            # mm1(b) | spT(b-2) | gT(b) | mm2(b-2) -- recon consumes results
            # produced two iterations earlier, so the PE never waits on the
            # cross-engine shrink chain or the ACT drain queue.
            pend = []
            for b in range(NB):
                gt_s = emit_mm1(b)
                st4 = emit_spT(pend[0][1]) if len(pend) >= 2 else None
                gsp4 = emit_gT(b, gt_s)
                if len(pend) >= 2:
                    emit_mm2(pend.pop(0)[0], st4)
                pend.append((b, emit_shrink(b, gsp4)))
            for bb, sps in pend:
                st4 = emit_spT(sps)
                emit_mm2(bb, st4)

    _split_multi_waits(nc, mybir)
    return nc


def _split_multi_waits(nc, mybir):
    """walrus codegen allows only one sync wait on most compute instruction
    structs (PE LDWEIGHTS, DVE TS, ...). Move the waits of any multi-wait
    compute instruction onto a NoOp inserted just before it: each engine's
    sequencer executes in order, so all waits still happen-before it."""
    skip = (
        mybir.InstNoOp,
        mybir.InstEventSemaphore,
        mybir.InstUnconditionalBranch,
        mybir.InstRegisterMove,
    )
    for f in nc.m.functions:
        for blk in f.blocks:
            insts = list(blk.instructions)
            out = []
            changed = False
            for ins in insts:
                si = getattr(ins, "sync_info", None)
                if (
                    not isinstance(ins, skip)
                    and getattr(ins, "engine", None) is not None
                    and si is not None
                    and si.on_wait
                    and len(si.on_wait) > 1
                ):
                    waits = list(si.on_wait)
                    for k, w in enumerate(waits[:-1]):
                        nop = mybir.InstNoOp(
                            name=f"{ins.name}-waitsplit{k}", ins=[], outs=[]
                        )
                        nop.engine = ins.engine
                        nop.sync_info = mybir.SyncInfo(
                            on_wait=[w], on_update=[]
                        )
                        out.append(nop)
                    ins.sync_info = mybir.SyncInfo(
                        on_wait=[waits[-1]], on_update=list(si.on_update)
                    )
                    changed = True
                out.append(ins)
            if changed:
                blk.instructions = out


def _prep_inputs(x, gates, alpha, tau, signs, perm, inv_perm, target_idx):
    """Host-side prep: shard + transpose + cast per core."""
    tidx = int(target_idx)
    signs = np.asarray(signs, dtype=np.float64)
    perm = np.asarray(perm, dtype=np.int64)
    inv_perm = np.asarray(inv_perm, dtype=np.int64)
    x = np.asarray(x)

    # Sense matrix A: row i = i-th output of FWHT(permute(e * signs))[:64].
    eye = np.eye(DIM, dtype=np.float64)
    A = _fwht((eye * signs[None, :])[:, perm])[:, :M].T          # [64, 1024]
    # Reconstruct matrix B (== A, but built independently for safety)
    pad = np.zeros((M, DIM), dtype=np.float64)
    pad[:, :M] = np.eye(M)
    B = _fwht(pad)[:, inv_perm] * signs[None, :]                 # [64, 1024]

    in_maps = []
    for c in range(NCORES):
        b, half = divmod(c, 2)
        g = np.asarray(gates, dtype=np.float64)[b, tidx]         # [64]
        al = float(np.asarray(alpha, dtype=np.float64)[b, tidx, 0])
        tu = abs(float(np.asarray(tau, dtype=np.float64)[b, tidx, 0]))
        a1 = g[:, None] * A                                      # [64, 1024]
        a1t = np.ascontiguousarray(
            a1.T.reshape(NC_, 128, M).transpose(1, 0, 2).reshape(128, NC_ * M)
        ).astype(np.float16)
        a2 = (al * B).astype(np.float16)                         # [64, 1024]
        xs = x[b, half * TOK:(half + 1) * TOK, :].astype(np.float16)
        # [tok, dim] -> [blk, p, c, t] -> [NB*128, NC_*BLK]
        xt = np.ascontiguousarray(
            xs.reshape(NB, BLK, NC_, 128).transpose(0, 3, 2, 1)
        ).reshape(NB * 128, NC_ * BLK)
        in_maps.append({
            "xt": xt,
            "a1t": a1t,
            "a2": np.ascontiguousarray(a2),
            "tau": np.full((128, 1), tu, dtype=np.float32),
        })
    return in_maps


def _get_nc():
    if "nc" not in _cache:
        _cache["nc"] = _build_nc()
    return _cache["nc"]


def kernel(x, gates, alpha, tau, signs, perm, inv_perm, target_idx,
           _trace=False, _tmpdir=None):
    from concourse.bass_utils import run_bass_kernel_spmd

    nc = _get_nc()
    in_maps = _prep_inputs(x, gates, alpha, tau, signs, perm, inv_perm,
                           target_idx)
    res = run_bass_kernel_spmd(
        nc, in_maps, core_ids=list(range(NCORES)),
        trace=_trace, tmpdir=_tmpdir,
    )
    if _trace:
        _cache["last_results"] = res
    out = np.empty((BSZ, SEQ, DIM), dtype=np.float32)
    for c in range(NCORES):
        b, half = divmod(c, 2)
        od = res.results[c]["od"]
        # [NB*128, NC_*BLK] -> [blk, p, c, t] -> [tok, dim]
        o = od.reshape(NB, 128, NC_, BLK).transpose(0, 3, 2, 1).reshape(
            TOK, DIM)
        out[b, half * TOK:(half + 1) * TOK, :] = o.astype(np.float32)
    return out
